# revision 1
# baseline (speedup 1.0000x reference)
"""AttentionDTI forward pass on 8 Trainium2 NeuronCores (pure data parallel).

Batch of 8 peptide/MHC pairs; one batch element per core, weights
replicated. The 4D additive-attention tensor h[b,p,m,c] =
relu(p_att + m_att) is never materialized in HBM: since the following
linear layer is, well, linear, mean_m(h @ Wa) == mean_m(h) @ Wa, so the
kernel only accumulates hp[c,p] = sum_m h and hm[c,m] = sum_p h on the
fly. hp comes from the ScalarEngine's fused relu+bias+accum activation;
hm is accumulated in PSUM by streaming h tiles through the TensorEngine
against a stationary identity matrix.

Environment constraints discovered empirically (this axon terminal):
  - GPSIMD/Pool ucode ops (SWDGE DMA, gpsimd memset/iota) hang: all DMAs
    go through the sync-engine HWDGE, memsets through the VectorEngine.
  - scalar_tensor_tensor hangs: only tensor_scalar / tensor_tensor /
    tensor_reduce / activation / matmul are used.
  - walrus here allows at most ONE semaphore wait per instruction:
    _split_excess_waits() rewrites the Tile-scheduled program, moving
    excess waits onto standalone InstEventSemaphore instructions.
"""
import sys

_BASS_ROOT = '/opt/trn_rl_repo'
if _BASS_ROOT not in sys.path:
    sys.path.insert(0, _BASS_ROOT)

import numpy as np
import ml_dtypes

import concourse.bass as bass
import concourse.tile as tile
from concourse import mybir
from concourse.bass_utils import run_bass_kernel_spmd

F32 = mybir.dt.float32
BF16 = mybir.dt.bfloat16
ALU = mybir.AluOpType
AF = mybir.ActivationFunctionType
AX = mybir.AxisListType

# model dims (hardcoded from the problem spec)
B = 8
LP, LM, DIM, CONV = 100, 1000, 64, 40
C2, C4 = CONV * 2, CONV * 4          # 80, 160
K1, K2, K3 = 4, 6, 8
LP1, LP2, LP3 = 97, 92, 85           # peptide conv output lengths
LM1, LM2, LM3 = 997, 992, 985        # MHC conv output lengths
MPAD = 992                           # LM3 padded to a multiple of 32
NEG = -30000.0                       # -inf stand-in that survives bf16
NPB = 22                             # ceil(85/4) packed p-groups for the c[128:160] chunk

_ctr = [0]


def _split_excess_waits(nc, max_waits=1):
    n_split = 0
    for f in nc.m.functions:
        for b in f.blocks:
            insts = list(b.instructions)
            out = []
            changed = False
            for inst in insts:
                si = inst.sync_info
                waits = list(si.on_wait) if (si is not None and si.on_wait) else []
                if len(waits) > max_waits:
                    changed = True
                    n_split += 1
                    keep = max(1, max_waits)
                    head, tail = waits[:-keep], waits[-keep:]
                    for i in range(0, len(head), keep):
                        chunk = head[i:i + keep]
                        nop = mybir.InstEventSemaphore(
                            name=f"ant-wait-split-{_ctr[0]}", ins=[], outs=[])
                        _ctr[0] += 1
                        nop.engine = inst.engine
                        nop.sync_info = mybir.SyncInfo(on_wait=chunk, on_update=[])
                        nc.register_instruction(nop)
                        out.append(nop)
                    upd = list(si.on_update) if si.on_update else []
                    inst.sync_info = mybir.SyncInfo(on_wait=tail, on_update=upd)
                out.append(inst)
            if changed:
                b.instructions = out
    return n_split


def _conv_matmuls(nc, psum, wtile, x, k_taps, co_lo, co_hi, m_lo, m_hi, cout_stride):
    """Accumulate a valid 1-D conv as k shifted matmuls into `psum`.

    psum: [co_hi-co_lo, m_hi-m_lo]; wtile: [ci, K*cout_stride] with tap k
    at columns [k*cout_stride, (k+1)*cout_stride); x: [ci, L].
    """
    for k in range(k_taps):
        nc.tensor.matmul(
            psum,
            wtile[:, k * cout_stride + co_lo: k * cout_stride + co_hi],
            x[:, m_lo + k: m_hi + k],
            start=(k == 0), stop=(k == k_taps - 1))


def _build_program():
    nc = bass.Bass("TRN2", target_bir_lowering=False, debug=False)

    def par(name, shape, dtype=F32):
        return nc.declare_dram_parameter(name, list(shape), dtype, isOutput=False)

    # per-core activations
    poh_e = par("pep_oh", [26, LP])
    moh_e = par("mhc_oh", [26, LM])
    # tables / weights (identical on all cores)
    pemb_e = par("pep_emb", [26, DIM])
    memb_e = par("mhc_emb", [26, DIM])
    pw1_e = par("pw1t", [DIM, K1 * CONV])
    pw2_e = par("pw2t", [CONV, K2 * C2])
    pw3_e = par("pw3t", [C2, K3 * C4])
    mw1_e = par("mw1t", [DIM, K1 * CONV])
    mw2_e = par("mw2t", [CONV, K2 * C2])
    mw3_e = par("mw3t", [C2, K3 * C4])
    pb1_e = par("pb1", [CONV, 1]); pb2_e = par("pb2", [C2, 1]); pb3_e = par("pb3", [128, 2])
    mb1_e = par("mb1", [CONV, 1]); mb2_e = par("mb2", [C2, 1]); mb3_e = par("mb3", [128, 2])
    wpaa_e = par("wpa_a", [128, C4]); wpab_e = par("wpa_b", [32, C4])
    wmaa_e = par("wma_a", [128, C4]); wmab_e = par("wma_b", [32, C4])
    wcaa_e = par("wca_a", [128, C4]); wcab_e = par("wca_b", [32, C4])   # Wa/985
    wmaa2_e = par("wma2_a", [128, C4]); wmab2_e = par("wma2_b", [32, C4])  # Wa/85
    bpa_e = par("bpa", [128, 2]); bma_e = par("bma", [128, 2]); ba_e = par("ba", [128, 2])
    w1a_e = par("w1a", [128, 2 * 1024]); w1b_e = par("w1b", [32, 2 * 1024])
    w2_e = par("w2", [128, 8 * 1024])
    w3_e = par("w3", [128, 8 * 512])
    wo_e = par("wo", [128, 8])
    b1_e = par("b1", [128, 8]); b2_e = par("b2", [128, 8]); b3_e = par("b3", [128, 4])
    bo_e = par("bo", [2, 1])
    id128_e = par("ident128", [128, 128], BF16)
    idst_e = par("ident_st", [128, 32], BF16)

    out_e = nc.declare_dram_parameter("out", [2, 1], F32, isOutput=True)

    with tile.TileContext(nc) as tc:
        with tc.tile_pool(name="consts", bufs=1) as cp, \
             tc.tile_pool(name="work", bufs=1) as wp, \
             tc.tile_pool(name="hpool", bufs=6) as hpool, \
             tc.tile_pool(name="ps_hm", bufs=1, space="PSUM") as ps_hm, \
             tc.tile_pool(name="ps_work", bufs=2, space="PSUM") as ps:

            def load(ext, shape, dtype=F32, name=None):
                t = cp.tile(shape, dtype, name=name or ext.name + "_sb")
                nc.sync.dma_start(out=t, in_=ext[:])
                return t

            # ---- constant loads (small, needed early) ----
            poh = load(poh_e, [26, LP]); moh = load(moh_e, [26, LM])
            pemb = load(pemb_e, [26, DIM]); memb = load(memb_e, [26, DIM])
            pw1 = load(pw1_e, [DIM, K1 * CONV]); pw2 = load(pw2_e, [CONV, K2 * C2]); pw3 = load(pw3_e, [C2, K3 * C4])
            mw1 = load(mw1_e, [DIM, K1 * CONV]); mw2 = load(mw2_e, [CONV, K2 * C2]); mw3 = load(mw3_e, [C2, K3 * C4])
            pb1 = load(pb1_e, [CONV, 1]); pb2 = load(pb2_e, [C2, 1]); pb3 = load(pb3_e, [128, 2])
            mb1 = load(mb1_e, [CONV, 1]); mb2 = load(mb2_e, [C2, 1]); mb3 = load(mb3_e, [128, 2])
            wpaa = load(wpaa_e, [128, C4]); wpab = load(wpab_e, [32, C4])
            wmaa = load(wmaa_e, [128, C4]); wmab = load(wmab_e, [32, C4])
            wcaa = load(wcaa_e, [128, C4]); wcab = load(wcab_e, [32, C4])
            wmaa2 = load(wmaa2_e, [128, C4]); wmab2 = load(wmab2_e, [32, C4])
            bpa = load(bpa_e, [128, 2]); bma = load(bma_e, [128, 2]); ba = load(ba_e, [128, 2])
            id128 = load(id128_e, [128, 128], BF16); idst = load(idst_e, [128, 32], BF16)
            # FC weights (big, only needed at the very end)
            w1a = load(w1a_e, [128, 2048]); w1b = load(w1b_e, [32, 2048])
            w2 = load(w2_e, [128, 8192])
            w3 = load(w3_e, [128, 4096])
            wo = load(wo_e, [128, 8])
            b1 = load(b1_e, [128, 8]); b2 = load(b2_e, [128, 8]); b3 = load(b3_e, [128, 4])
            bo = load(bo_e, [2, 1])

            # ---- embeddings: emb.T @ onehot -> [DIM, L] ----
            pe_ps = ps.tile([DIM, LP], F32, name="pe_ps", tag="ps")
            nc.tensor.matmul(pe_ps, pemb, poh, start=True, stop=True)
            pe = wp.tile([DIM, LP], F32, name="pe")
            nc.scalar.copy(pe, pe_ps)

            me_ps = ps.tile([DIM, LM], F32, name="me_ps", tag="ps")
            nc.tensor.matmul(me_ps[:, 0:512], pemb if False else memb, moh[:, 0:512], start=True, stop=True)
            nc.tensor.matmul(me_ps[:, 512:LM], memb, moh[:, 512:LM], start=True, stop=True)
            me = wp.tile([DIM, LM], F32, name="me")
            nc.scalar.copy(me, me_ps)

            # ---- peptide conv stack ----
            px1_ps = ps.tile([CONV, LP1], F32, name="px1_ps", tag="ps")
            _conv_matmuls(nc, px1_ps, pw1, pe, K1, 0, CONV, 0, LP1, CONV)
            px1 = wp.tile([CONV, LP1], F32, name="px1")
            nc.scalar.activation(out=px1, in_=px1_ps, func=AF.Relu, bias=pb1[:, 0:1])

            px2_ps = ps.tile([C2, LP2], F32, name="px2_ps", tag="ps")
            _conv_matmuls(nc, px2_ps, pw2, px1, K2, 0, C2, 0, LP2, C2)
            px2 = wp.tile([C2, LP2], F32, name="px2")
            nc.scalar.activation(out=px2, in_=px2_ps, func=AF.Relu, bias=pb2[:, 0:1])

            pc0_ps = ps.tile([128, LP3], F32, name="pc0_ps", tag="ps")
            _conv_matmuls(nc, pc0_ps, pw3, px2, K3, 0, 128, 0, LP3, C4)
            pc0 = wp.tile([128, LP3], F32, name="pc0")
            nc.scalar.activation(out=pc0, in_=pc0_ps, func=AF.Relu, bias=pb3[:, 0:1])
            pc1_ps = ps.tile([32, LP3], F32, name="pc1_ps", tag="ps")
            _conv_matmuls(nc, pc1_ps, pw3, px2, K3, 128, C4, 0, LP3, C4)
            pc1 = wp.tile([32, LP3], F32, name="pc1")
            nc.scalar.activation(out=pc1, in_=pc1_ps, func=AF.Relu, bias=pb3[0:32, 1:2])

            # ---- MHC conv stack (free dim chunked to <=512) ----
            mx1_ps = ps.tile([CONV, LM1], F32, name="mx1_ps", tag="ps")
            _conv_matmuls(nc, mx1_ps[:, 0:512], mw1, me, K1, 0, CONV, 0, 512, CONV)
            _conv_matmuls(nc, mx1_ps[:, 512:LM1], mw1, me, K1, 0, CONV, 512, LM1, CONV)
            mx1 = wp.tile([CONV, LM1], F32, name="mx1")
            nc.scalar.activation(out=mx1, in_=mx1_ps, func=AF.Relu, bias=mb1[:, 0:1])

            mx2_ps = ps.tile([C2, LM2], F32, name="mx2_ps", tag="ps")
            _conv_matmuls(nc, mx2_ps[:, 0:512], mw2, mx1, K2, 0, C2, 0, 512, C2)
            _conv_matmuls(nc, mx2_ps[:, 512:LM2], mw2, mx1, K2, 0, C2, 512, LM2, C2)
            mx2 = wp.tile([C2, LM2], F32, name="mx2")
            nc.scalar.activation(out=mx2, in_=mx2_ps, func=AF.Relu, bias=mb2[:, 0:1])

            mc0_ps = ps.tile([128, LM3], F32, name="mc0_ps", tag="ps")
            _conv_matmuls(nc, mc0_ps[:, 0:512], mw3, mx2, K3, 0, 128, 0, 512, C4)
            _conv_matmuls(nc, mc0_ps[:, 512:LM3], mw3, mx2, K3, 0, 128, 512, LM3, C4)
            mc0 = wp.tile([128, LM3], F32, name="mc0")
            nc.scalar.activation(out=mc0, in_=mc0_ps, func=AF.Relu, bias=mb3[:, 0:1])
            mc1_ps = ps.tile([32, LM3], F32, name="mc1_ps", tag="ps")
            _conv_matmuls(nc, mc1_ps[:, 0:512], mw3, mx2, K3, 128, C4, 0, 512, C4)
            _conv_matmuls(nc, mc1_ps[:, 512:LM3], mw3, mx2, K3, 128, C4, 512, LM3, C4)
            mc1 = wp.tile([32, LM3], F32, name="mc1")
            nc.scalar.activation(out=mc1, in_=mc1_ps, func=AF.Relu, bias=mb3[0:32, 1:2])

            # ---- attention projections ----
            # pa[c,p] = sum_c' pc[c',p] * Wpa[c',c] + bpa[c]
            pa0_ps = ps.tile([128, LP3], F32, name="pa0_ps", tag="ps")
            nc.tensor.matmul(pa0_ps, wpaa[:, 0:128], pc0, start=True, stop=False)
            nc.tensor.matmul(pa0_ps, wpab[:, 0:128], pc1, start=False, stop=True)
            pa0 = wp.tile([128, LP3], F32, name="pa0")
            nc.scalar.add(pa0, pa0_ps, bpa[:, 0:1])

            pa1_ps = ps.tile([32, LP3], F32, name="pa1_ps", tag="ps")
            nc.tensor.matmul(pa1_ps, wpaa[:, 128:C4], pc0, start=True, stop=False)
            nc.tensor.matmul(pa1_ps, wpab[:, 128:C4], pc1, start=False, stop=True)
            pa1 = wp.tile([32, 4 * NPB], F32, name="pa1")
            nc.vector.memset(pa1, NEG)
            nc.scalar.add(pa1[:, 0:LP3], pa1_ps, bpa[0:32, 1:2])
            # pack 4 p-positions per 32-row block: pa1p[32j+d, g] = pa1[d, 4g+j]
            pa1p = wp.tile([128, NPB], F32, name="pa1p")
            pa1_g = pa1.rearrange("d (g f) -> d g f", f=4)
            for j in range(4):
                nc.sync.dma_start(
                    out=pa1p[32 * j:32 * (j + 1), 0:NPB],
                    in_=pa1_g[:, :, j])

            # ma[c,m] = sum_c' mc[c',m] * Wma[c',c] + bma[c]  (bf16, m padded with NEG)
            ma0_ps = ps.tile([128, LM3], F32, name="ma0_ps", tag="ps")
            nc.tensor.matmul(ma0_ps[:, 0:512], wmaa[:, 0:128], mc0[:, 0:512], start=True, stop=False)
            nc.tensor.matmul(ma0_ps[:, 0:512], wmab[:, 0:128], mc1[:, 0:512], start=False, stop=True)
            nc.tensor.matmul(ma0_ps[:, 512:LM3], wmaa[:, 0:128], mc0[:, 512:LM3], start=True, stop=False)
            nc.tensor.matmul(ma0_ps[:, 512:LM3], wmab[:, 0:128], mc1[:, 512:LM3], start=False, stop=True)
            ma0 = wp.tile([128, MPAD], BF16, name="ma0")
            nc.vector.memset(ma0, NEG)
            nc.scalar.add(ma0[:, 0:LM3], ma0_ps, bma[:, 0:1])

            ma1_ps = ps.tile([32, LM3], F32, name="ma1_ps", tag="ps")
            nc.tensor.matmul(ma1_ps[:, 0:512], wmaa[:, 128:C4], mc0[:, 0:512], start=True, stop=False)
            nc.tensor.matmul(ma1_ps[:, 0:512], wmab[:, 128:C4], mc1[:, 0:512], start=False, stop=True)
            nc.tensor.matmul(ma1_ps[:, 512:LM3], wmaa[:, 128:C4], mc0[:, 512:LM3], start=True, stop=False)
            nc.tensor.matmul(ma1_ps[:, 512:LM3], wmab[:, 128:C4], mc1[:, 512:LM3], start=False, stop=True)
            ma1 = wp.tile([32, MPAD], BF16, name="ma1")
            nc.vector.memset(ma1, NEG)
            nc.scalar.add(ma1[:, 0:LM3], ma1_ps, bma[0:32, 1:2])
            # replicate 4x vertically for the packed c[128:160] loop
            ma1p = wp.tile([128, MPAD], BF16, name="ma1p")
            for j in range(4):
                nc.sync.dma_start(out=ma1p[32 * j:32 * (j + 1), :], in_=ma1[:, :])

            # ---- 4D attention reductions ----
            hp0 = wp.tile([128, 88], F32, name="hp0")
            hp1p = wp.tile([128, NPB], F32, name="hp1p")
            hm0_ps = ps_hm.tile([128, MPAD], F32, name="hm0_ps")
            hm1_ps = ps_hm.tile([32, MPAD], F32, name="hm1_ps")

            for p in range(LP3):
                h = hpool.tile([128, MPAD], BF16, tag="h", name="h")
                nc.scalar.activation(out=h, in_=ma0, func=AF.Relu,
                                     bias=pa0[:, p:p + 1], accum_out=hp0[:, p:p + 1])
                nc.tensor.matmul(hm0_ps[:, 0:512], id128, h[:, 0:512],
                                 start=(p == 0), stop=(p == LP3 - 1))
                nc.tensor.matmul(hm0_ps[:, 512:MPAD], id128, h[:, 512:MPAD],
                                 start=(p == 0), stop=(p == LP3 - 1))
            for g in range(NPB):
                h = hpool.tile([128, MPAD], BF16, tag="h", name="h")
                nc.scalar.activation(out=h, in_=ma1p, func=AF.Relu,
                                     bias=pa1p[:, g:g + 1], accum_out=hp1p[:, g:g + 1])
                nc.tensor.matmul(hm1_ps[:, 0:512], idst, h[:, 0:512],
                                 start=(g == 0), stop=(g == NPB - 1))
                nc.tensor.matmul(hm1_ps[:, 512:MPAD], idst, h[:, 512:MPAD],
                                 start=(g == 0), stop=(g == NPB - 1))

            # unpack hp1p -> hp1[d, 4g+j]
            hp1 = wp.tile([32, 88], F32, name="hp1")
            hp1_g = hp1.rearrange("d (g f) -> d g f", f=4)
            for j in range(4):
                nc.sync.dma_start(
                    out=hp1_g[:, :, j],
                    in_=hp1p[32 * j:32 * j + 32, 0:NPB])

            # ---- peptide attention gate ----
            # catt[d,p] = sigmoid(sum_c hp[c,p]/LM3 * Wa[c,d] + ba[d]); 1/LM3 folded into wca
            cl0_ps = ps.tile([128, LP3], F32, name="cl0_ps", tag="ps")
            nc.tensor.matmul(cl0_ps, wcaa[:, 0:128], hp0[:, 0:LP3], start=True, stop=False)
            nc.tensor.matmul(cl0_ps, wcab[:, 0:128], hp1[:, 0:LP3], start=False, stop=True)
            catt0 = wp.tile([128, LP3], F32, name="catt0")
            nc.scalar.activation(out=catt0, in_=cl0_ps, func=AF.Sigmoid, bias=ba[:, 0:1])
            cl1_ps = ps.tile([32, LP3], F32, name="cl1_ps", tag="ps")
            nc.tensor.matmul(cl1_ps, wcaa[:, 128:C4], hp0[:, 0:LP3], start=True, stop=False)
            nc.tensor.matmul(cl1_ps, wcab[:, 128:C4], hp1[:, 0:LP3], start=False, stop=True)
            catt1 = wp.tile([32, LP3], F32, name="catt1")
            nc.scalar.activation(out=catt1, in_=cl1_ps, func=AF.Sigmoid, bias=ba[0:32, 1:2])

            pg0 = wp.tile([128, LP3], F32, name="pg0")
            nc.vector.tensor_scalar(out=catt0, in0=catt0, scalar1=0.5, scalar2=None, op0=ALU.add)
            nc.vector.tensor_tensor(out=pg0, in0=catt0, in1=pc0, op=ALU.mult)
            pv0 = wp.tile([128, 1], F32, name="pv0")
            nc.vector.tensor_reduce(out=pv0, in_=pg0, op=ALU.max, axis=AX.X)
            pg1 = wp.tile([32, LP3], F32, name="pg1")
            nc.vector.tensor_scalar(out=catt1, in0=catt1, scalar1=0.5, scalar2=None, op0=ALU.add)
            nc.vector.tensor_tensor(out=pg1, in0=catt1, in1=pc1, op=ALU.mult)
            pv1 = wp.tile([32, 1], F32, name="pv1")
            nc.vector.tensor_reduce(out=pv1, in_=pg1, op=ALU.max, axis=AX.X)

            # ---- MHC attention gate ----
            hm0 = wp.tile([128, LM3], F32, name="hm0")
            nc.scalar.copy(hm0, hm0_ps[:, 0:LM3])
            hm1 = wp.tile([32, LM3], F32, name="hm1")
            nc.scalar.copy(hm1, hm1_ps[:, 0:LM3])

            matt0 = wp.tile([128, LM3], F32, name="matt0")
            ml0_ps = ps.tile([128, LM3], F32, name="ml0_ps", tag="ps")
            for lo, hi in ((0, 512), (512, LM3)):
                nc.tensor.matmul(ml0_ps[:, lo:hi], wmaa2[:, 0:128], hm0[:, lo:hi], start=True, stop=False)
                nc.tensor.matmul(ml0_ps[:, lo:hi], wmab2[:, 0:128], hm1[:, lo:hi], start=False, stop=True)
            nc.scalar.activation(out=matt0, in_=ml0_ps, func=AF.Sigmoid, bias=ba[:, 0:1])
            matt1 = wp.tile([32, LM3], F32, name="matt1")
            ml1_ps = ps.tile([32, LM3], F32, name="ml1_ps", tag="ps")
            for lo, hi in ((0, 512), (512, LM3)):
                nc.tensor.matmul(ml1_ps[:, lo:hi], wmaa2[:, 128:C4], hm0[:, lo:hi], start=True, stop=False)
                nc.tensor.matmul(ml1_ps[:, lo:hi], wmab2[:, 128:C4], hm1[:, lo:hi], start=False, stop=True)
            nc.scalar.activation(out=matt1, in_=ml1_ps, func=AF.Sigmoid, bias=ba[0:32, 1:2])

            mg0 = wp.tile([128, LM3], F32, name="mg0")
            nc.vector.tensor_scalar(out=matt0, in0=matt0, scalar1=0.5, scalar2=None, op0=ALU.add)
            nc.vector.tensor_tensor(out=mg0, in0=matt0, in1=mc0, op=ALU.mult)
            mv0 = wp.tile([128, 1], F32, name="mv0")
            nc.vector.tensor_reduce(out=mv0, in_=mg0, op=ALU.max, axis=AX.X)
            mg1 = wp.tile([32, LM3], F32, name="mg1")
            nc.vector.tensor_scalar(out=matt1, in0=matt1, scalar1=0.5, scalar2=None, op0=ALU.add)
            nc.vector.tensor_tensor(out=mg1, in0=matt1, in1=mc1, op=ALU.mult)
            mv1 = wp.tile([32, 1], F32, name="mv1")
            nc.vector.tensor_reduce(out=mv1, in_=mg1, op=ALU.max, axis=AX.X)

            # ---- FC head: outputs laid out [128 partitions, block] ----
            def fc_layer(name, w_tiles_rhs, nblk, blk_w, bias, nout_cols):
                """w_tiles_rhs: list of (wtile, col_base, rhs [K,1]) accumulated per block."""
                f_ps = ps.tile([128, nout_cols], F32, name=name + "_ps", tag="ps")
                for a in range(nblk):
                    n = len(w_tiles_rhs)
                    for i, (wt, base, rhs) in enumerate(w_tiles_rhs):
                        nc.tensor.matmul(
                            f_ps[:, a:a + 1],
                            wt[:, base + a * 128: base + a * 128 + 128],
                            rhs,
                            start=(i == 0), stop=(i == n - 1))
                fb = wp.tile([128, nout_cols], F32, name=name + "_b")
                nc.vector.tensor_tensor(out=fb, in0=f_ps, in1=bias, op=ALU.add)
                fs = wp.tile([128, nout_cols], F32, name=name + "_s")
                nc.vector.tensor_scalar(out=fs, in0=fb, scalar1=0.01, scalar2=None, op0=ALU.mult)
                fo = wp.tile([128, nout_cols], F32, name=name)
                nc.vector.tensor_tensor(out=fo, in0=fb, in1=fs, op=ALU.max)
                return fo

            f1 = fc_layer("f1", [(w1a, 0, pv0), (w1b, 0, pv1), (w1a, 1024, mv0), (w1b, 1024, mv1)],
                          8, 128, b1, 8)
            f2 = fc_layer("f2", [(w2, jb * 1024, f1[:, jb:jb + 1]) for jb in range(8)], 8, 128, b2, 8)
            f3 = fc_layer("f3", [(w3, jb * 512, f2[:, jb:jb + 1]) for jb in range(8)], 4, 128, b3, 4)

            o_ps = ps.tile([2, 1], F32, name="o_ps", tag="ps")
            for c in range(4):
                nc.tensor.matmul(o_ps, wo[:, 2 * c:2 * c + 2], f3[:, c:c + 1],
                                 start=(c == 0), stop=(c == 3))
            o_sb = wp.tile([2, 1], F32, name="o_sb")
            nc.vector.tensor_tensor(out=o_sb, in0=o_ps, in1=bo, op=ALU.add)
            nc.sync.dma_start(out=out_e[:], in_=o_sb)

    _split_excess_waits(nc, max_waits=1)
    return nc


_PROGRAM = None


def _get_program():
    global _PROGRAM
    if _PROGRAM is None:
        _PROGRAM = _build_program()
    return _PROGRAM


def _prep_weights(inp):
    """Host-side layout prep shared by all cores. All f32 contiguous."""
    f = lambda x: np.ascontiguousarray(np.asarray(x, dtype=np.float32))
    bf = lambda x: np.ascontiguousarray(np.asarray(x).astype(ml_dtypes.bfloat16))

    def convw(w):  # [co, ci, k] -> [ci, k*co]
        w = np.asarray(w, dtype=np.float32)
        ci = w.shape[1]
        return np.ascontiguousarray(w.transpose(1, 2, 0).reshape(ci, -1))

    def bias2(b):  # [160] -> [128, 2] (col 0 = [0:128], col 1 rows 0:32 = [128:160])
        b = np.asarray(b, dtype=np.float32)
        out = np.zeros((128, 2), np.float32)
        out[:, 0] = b[0:128]
        out[0:32, 1] = b[128:160]
        return out

    def fcw(w, nblk):  # [I, J] with I = nblk*128 -> [128, nblk*J]
        w = np.asarray(w, dtype=np.float32)
        i, j = w.shape
        return np.ascontiguousarray(w.reshape(nblk, 128, j).transpose(1, 0, 2).reshape(128, nblk * j))

    def fcb(b, nblk):  # [nblk*128] -> [128, nblk]
        b = np.asarray(b, dtype=np.float32)
        return np.ascontiguousarray(b.reshape(nblk, 128).T)

    wa985 = np.asarray(inp['Wa'], np.float32) / float(LM3)
    wa85 = np.asarray(inp['Wa'], np.float32) / float(LP3)
    w1 = np.asarray(inp['W1'], np.float32)
    d = {
        'pep_emb': f(inp['pep_emb']), 'mhc_emb': f(inp['mhc_emb']),
        'pw1t': convw(inp['pw1']), 'pw2t': convw(inp['pw2']), 'pw3t': convw(inp['pw3']),
        'mw1t': convw(inp['mw1']), 'mw2t': convw(inp['mw2']), 'mw3t': convw(inp['mw3']),
        'pb1': f(inp['pb1']).reshape(CONV, 1), 'pb2': f(inp['pb2']).reshape(C2, 1), 'pb3': bias2(inp['pb3']),
        'mb1': f(inp['mb1']).reshape(CONV, 1), 'mb2': f(inp['mb2']).reshape(C2, 1), 'mb3': bias2(inp['mb3']),
        'wpa_a': f(inp['Wpa'][0:128]), 'wpa_b': f(inp['Wpa'][128:160]),
        'wma_a': f(inp['Wma'][0:128]), 'wma_b': f(inp['Wma'][128:160]),
        'wca_a': f(wa985[0:128]), 'wca_b': f(wa985[128:160]),
        'wma2_a': f(wa85[0:128]), 'wma2_b': f(wa85[128:160]),
        'bpa': bias2(inp['bpa']), 'bma': bias2(inp['bma']), 'ba': bias2(inp['ba']),
        'w1a': np.ascontiguousarray(np.concatenate([w1[0:128], w1[160:288]], axis=1)),
        'w1b': np.ascontiguousarray(np.concatenate([w1[128:160], w1[288:320]], axis=1)),
        'w2': fcw(inp['W2'], 8), 'w3': fcw(inp['W3'], 8),
        'wo': fcw(inp['Wo'], 4),
        'b1': fcb(inp['b1'], 8), 'b2': fcb(inp['b2'], 8),
        'b3': np.ascontiguousarray(np.asarray(inp['b3'], np.float32).reshape(4, 128).T),
        'bo': f(inp['bo']).reshape(2, 1),
        'ident128': np.eye(128, dtype=ml_dtypes.bfloat16),
        'ident_st': np.ascontiguousarray(np.tile(np.eye(32, dtype=ml_dtypes.bfloat16), (4, 1))),
    }
    return d


def _onehot(idx, length):
    idx = np.asarray(idx).astype(np.int64)
    oh = np.zeros((26, length), np.float32)
    oh[idx, np.arange(length)] = 1.0
    return oh


def kernel(**inputs):
    nc = _get_program()
    shared = _prep_weights(inputs)
    peptide = np.asarray(inputs['peptide'])
    mhc = np.asarray(inputs['MHC'])
    in_maps = []
    for b in range(B):
        m = dict(shared)
        m['pep_oh'] = _onehot(peptide[b], LP)
        m['mhc_oh'] = _onehot(mhc[b], LM)
        in_maps.append(m)
    res = run_bass_kernel_spmd(nc, in_maps, core_ids=list(range(B)))
    return np.stack([np.asarray(res.results[i]['out']).reshape(2) for i in range(B)]).astype(np.float32)



# revision 6
# speedup vs baseline: 2.0928x; 2.0928x over previous
"""AttentionDTI forward pass on 8 Trainium2 NeuronCores (pure data parallel).

Batch of 8 peptide/MHC pairs; one batch element per core, weights
replicated. The 4D additive-attention tensor h[b,p,m,c] =
relu(p_att + m_att) is never materialized in HBM: since the following
linear layer is, well, linear, mean_m(h @ Wa) == mean_m(h) @ Wa, so the
kernel only accumulates hp[c,p] = sum_m h and hm[c,m] = sum_p h on the
fly.

v2 layout of the work:
  - h planes are produced by the VectorEngine's fused
    tensor_scalar(add bias, max 0, accum_out) — one DVE op per plane at
    the 4x bf16 rate, with the free-axis sum (hp) falling out of the
    accumulator.
  - hm = sum_p h is accumulated in PSUM by streaming h tiles through the
    TensorEngine against a stationary identity; a tunable fraction of
    plane pairs is pre-summed on the DVE (tensor_tensor add) to halve
    the PE stream for those planes, balancing DVE vs PE time.
  - all PE moving operands are bf16 (fp32 moving runs at half rate).
  - the FC head uses the vector-stationary trick: out[1,N] = x.T @ W
    with W as the bf16 moving operand (killing the per-column matmul +
    LDWEIGHTS storm), bias rows folded in via a ones-stationary matmul,
    Lrelu fused into the PSUM->SBUF copy, and [1,N]->[128,N/128]
    transposes done as tiny 1-row-stationary matmuls.

Environment constraints discovered empirically (this axon terminal):
  - GPSIMD/Pool ucode ops (SWDGE DMA, gpsimd memset/iota) hang: all DMAs
    go through the sync-engine HWDGE, memsets through the VectorEngine.
  - scalar_tensor_tensor hangs: only tensor_scalar / tensor_tensor /
    tensor_reduce / activation / matmul are used.
  - walrus here allows at most ONE semaphore wait per instruction:
    _split_excess_waits() rewrites the Tile-scheduled program, moving
    excess waits onto standalone InstEventSemaphore instructions.
"""
import sys

_BASS_ROOT = '/opt/trn_rl_repo'
if _BASS_ROOT not in sys.path:
    sys.path.insert(0, _BASS_ROOT)

import numpy as np
import ml_dtypes

import concourse.bass as bass
import concourse.tile as tile
from concourse import mybir
from concourse.bass_utils import run_bass_kernel_spmd

F32 = mybir.dt.float32
F16 = mybir.dt.float16
ALU = mybir.AluOpType
AF = mybir.ActivationFunctionType
AX = mybir.AxisListType

# model dims (hardcoded from the problem spec)
B = 8
LP, LM, DIM, CONV = 100, 1000, 64, 40
C2, C4 = CONV * 2, CONV * 4          # 80, 160
K1, K2, K3 = 4, 6, 8
LP1, LP2, LP3 = 97, 92, 85           # peptide conv output lengths
LM1, LM2, LM3 = 997, 992, 985        # MHC conv output lengths
MPAD = 992                           # LM3 padded to a multiple of 32
NEG = -30000.0                       # -inf stand-in that survives bf16
NPB = 22                             # ceil(85/4) packed p-groups for the c[128:160] chunk

# h-plane work split (measured rates per [128,992] bf16 plane:
# ACT fused relu+bias+accum 1.20us, DVE scalar_tensor_tensor fused
# 0.785us, DVE pair-add 0.67us, PE identity matmul 992 cycles):
N_ACT = 50    # loop-1 planes produced on the ScalarEngine (rest on DVE)
PAIRS1 = 25   # ACT plane pairs pre-summed on DVE before the PE reduction
PAIRS2 = 0    # pairs among the 22 packed c[128:160] tiles

_ctr = [0]


def _split_excess_waits(nc, max_waits=1):
    n_split = 0
    for f in nc.m.functions:
        for b in f.blocks:
            insts = list(b.instructions)
            out = []
            changed = False
            for inst in insts:
                si = inst.sync_info
                waits = list(si.on_wait) if (si is not None and si.on_wait) else []
                if len(waits) > max_waits:
                    changed = True
                    n_split += 1
                    keep = max(1, max_waits)
                    head, tail = waits[:-keep], waits[-keep:]
                    for i in range(0, len(head), keep):
                        chunk = head[i:i + keep]
                        nop = mybir.InstEventSemaphore(
                            name=f"ant-wait-split-{_ctr[0]}", ins=[], outs=[])
                        _ctr[0] += 1
                        nop.engine = inst.engine
                        nop.sync_info = mybir.SyncInfo(on_wait=chunk, on_update=[])
                        nc.register_instruction(nop)
                        out.append(nop)
                    upd = list(si.on_update) if si.on_update else []
                    inst.sync_info = mybir.SyncInfo(on_wait=tail, on_update=upd)
                out.append(inst)
            if changed:
                b.instructions = out
    return n_split


def _conv_matmuls(nc, psum, wtile, x, k_taps, co_lo, co_hi, m_lo, m_hi, cout_stride):
    """Accumulate a valid 1-D conv as k shifted matmuls into `psum`.

    psum: [co_hi-co_lo, m_hi-m_lo]; wtile: [ci, K*cout_stride] with tap k
    at columns [k*cout_stride, (k+1)*cout_stride); x: [ci, L].
    """
    for k in range(k_taps):
        nc.tensor.matmul(
            psum,
            wtile[:, k * cout_stride + co_lo: k * cout_stride + co_hi],
            x[:, m_lo + k: m_hi + k],
            start=(k == 0), stop=(k == k_taps - 1))


def _h_plan():
    """Merged emission plan for the 107 h reduction tiles.

    Entries: (loop_id, engine, (p, ...)) — loop 1 = c[0:128] planes feeding
    hm0_ps, loop 2 = packed c[128:160] tiles feeding hm1_ps. ACT pairs are
    pre-summed on DVE. Streams are interleaved so ACT/DVE/PE all stay busy.
    """
    a_groups = []   # ACT-produced loop-1 planes (paired where possible)
    p = 0
    for _ in range(PAIRS1):
        a_groups.append((1, 'act', (p, p + 1)))
        p += 2
    while p < N_ACT:
        a_groups.append((1, 'act', (p,)))
        p += 1
    d_groups = [(1, 'dve', (p,)) for p in range(N_ACT, LP3)]
    g = 0
    for _ in range(PAIRS2):
        d_groups.append((2, 'dve', (g, g + 1)))
        g += 2
    while g < NPB:
        d_groups.append((2, 'dve', (g,)))
        g += 1
    merged = []
    ai, di = 0, 0
    while ai < len(a_groups) or di < len(d_groups):
        if ai < len(a_groups):
            merged.append(a_groups[ai]); ai += 1
        for _ in range(2):
            if di < len(d_groups):
                merged.append(d_groups[di]); di += 1
    return merged


def _build_program():
    nc = bass.Bass("TRN2", target_bir_lowering=False, debug=False)

    def par(name, shape, dtype=F32):
        return nc.declare_dram_parameter(name, list(shape), dtype, isOutput=False)

    # per-core activations
    poh_e = par("pep_oh", [26, LP], F16)
    moh_e = par("mhc_oh", [26, LM], F16)
    # tables / weights (identical on all cores)
    pemb_e = par("pep_emb", [26, DIM], F16)
    memb_e = par("mhc_emb", [26, DIM], F16)
    pw1_e = par("pw1t", [DIM, K1 * CONV], F16)
    pw2_e = par("pw2t", [CONV, K2 * C2], F16)
    pw3_e = par("pw3t", [C2, K3 * C4], F16)
    mw1_e = par("mw1t", [DIM, K1 * CONV], F16)
    mw2_e = par("mw2t", [CONV, K2 * C2], F16)
    mw3_e = par("mw3t", [C2, K3 * C4], F16)
    pb1_e = par("pb1", [CONV, 1]); pb2_e = par("pb2", [C2, 1]); pb3_e = par("pb3", [128, 2])
    mb1_e = par("mb1", [CONV, 1]); mb2_e = par("mb2", [C2, 1]); mb3_e = par("mb3", [128, 2])
    wpaa_e = par("wpa_a", [128, C4], F16); wpab_e = par("wpa_b", [32, C4], F16)
    wmaa_e = par("wma_a", [128, C4], F16); wmab_e = par("wma_b", [32, C4], F16)
    wcaa_e = par("wca_a", [128, C4]); wcab_e = par("wca_b", [32, C4])   # Wa/985 (fp32: moving hp is fp32)
    wmaa2_e = par("wma2_a", [128, C4], F16); wmab2_e = par("wma2_b", [32, C4], F16)  # Wa/85
    bpa_e = par("bpa", [128, 2]); bma_e = par("bma", [128, 2]); ba_e = par("ba", [128, 2])
    # FC head: W chunks as bf16 moving operands; bias rows on one partition
    w1a_e = par("w1a", [128, 2 * 1024], F16)   # [W1[0:128,:] | W1[160:288,:]]
    w1b_e = par("w1b", [32, 2 * 1024], F16)    # [W1[128:160,:] | W1[288:320,:]]
    w2_e = par("w2", [128, 8 * 1024], F16)     # block c = W2[128c:128c+128, :]
    w3_e = par("w3", [128, 8 * 512], F16)
    wo_e = par("wo", [128, 4 * 2], F16)
    b1r_e = par("b1row", [1, 1024], F16)
    b2r_e = par("b2row", [1, 1024], F16)
    b3r_e = par("b3row", [1, 512], F16)
    bo_e = par("bo", [1, 2])
    id128_e = par("ident128", [128, 128], F16)
    idst_e = par("ident_st", [128, 32], F16)

    out_e = nc.declare_dram_parameter("out", [1, 2], F32, isOutput=True)

    with tile.TileContext(nc) as tc:
        with tc.tile_pool(name="consts", bufs=1) as cp, \
             tc.tile_pool(name="work", bufs=1) as wp, \
             tc.tile_pool(name="hpool", bufs=6) as hpool, \
             tc.tile_pool(name="hppool", bufs=3) as hppool, \
             tc.tile_pool(name="ps_hm", bufs=1, space="PSUM") as ps_hm, \
             tc.tile_pool(name="ps_work", bufs=2, space="PSUM") as ps:

            def load(ext, shape, dtype=F32, name=None):
                t = cp.tile(shape, dtype, name=name or ext.name + "_sb")
                nc.sync.dma_start(out=t, in_=ext[:])
                return t

            # ---- constant loads (small, needed early) ----
            poh = load(poh_e, [26, LP], F16); moh = load(moh_e, [26, LM], F16)
            pemb = load(pemb_e, [26, DIM], F16); memb = load(memb_e, [26, DIM], F16)
            pw1 = load(pw1_e, [DIM, K1 * CONV], F16); pw2 = load(pw2_e, [CONV, K2 * C2], F16)
            pw3 = load(pw3_e, [C2, K3 * C4], F16)
            mw1 = load(mw1_e, [DIM, K1 * CONV], F16); mw2 = load(mw2_e, [CONV, K2 * C2], F16)
            mw3 = load(mw3_e, [C2, K3 * C4], F16)
            pb1 = load(pb1_e, [CONV, 1]); pb2 = load(pb2_e, [C2, 1]); pb3 = load(pb3_e, [128, 2])
            mb1 = load(mb1_e, [CONV, 1]); mb2 = load(mb2_e, [C2, 1]); mb3 = load(mb3_e, [128, 2])
            wpaa = load(wpaa_e, [128, C4], F16); wpab = load(wpab_e, [32, C4], F16)
            wmaa = load(wmaa_e, [128, C4], F16); wmab = load(wmab_e, [32, C4], F16)
            wcaa = load(wcaa_e, [128, C4]); wcab = load(wcab_e, [32, C4])
            wmaa2 = load(wmaa2_e, [128, C4], F16); wmab2 = load(wmab2_e, [32, C4], F16)
            bpa = load(bpa_e, [128, 2]); bma = load(bma_e, [128, 2]); ba = load(ba_e, [128, 2])
            id128 = load(id128_e, [128, 128], F16); idst = load(idst_e, [128, 32], F16)
            # FC weights (big, only needed at the very end)
            w1a = load(w1a_e, [128, 2048], F16); w1b = load(w1b_e, [32, 2048], F16)
            w2 = load(w2_e, [128, 8192], F16)
            w3 = load(w3_e, [128, 4096], F16)
            wo = load(wo_e, [128, 8], F16)
            b1r = load(b1r_e, [1, 1024], F16); b2r = load(b2r_e, [1, 1024], F16)
            b3r = load(b3r_e, [1, 512], F16)
            bo = load(bo_e, [1, 2])

            one_bf = cp.tile([1, 1], F16, name="one_bf")
            nc.vector.memset(one_bf, 1.0)

            # ---- embeddings: emb.T @ onehot -> [DIM, L] (bf16) ----
            me_ps = ps.tile([DIM, LM], F32, name="me_ps", tag="ps")
            nc.tensor.matmul(me_ps[:, 0:512], memb, moh[:, 0:512], start=True, stop=True)
            nc.tensor.matmul(me_ps[:, 512:LM], memb, moh[:, 512:LM], start=True, stop=True)
            me = wp.tile([DIM, LM], F16, name="me")
            nc.scalar.copy(me, me_ps)

            pe_ps = ps.tile([DIM, LP], F32, name="pe_ps", tag="ps")
            nc.tensor.matmul(pe_ps, pemb, poh, start=True, stop=True)
            pe = wp.tile([DIM, LP], F16, name="pe")
            nc.scalar.copy(pe, pe_ps)

            # ---- conv stacks (MHC free dim chunked to <=512 fp32 PSUM cols) ----
            mx1_ps = ps.tile([CONV, LM1], F32, name="mx1_ps", tag="ps")
            _conv_matmuls(nc, mx1_ps[:, 0:512], mw1, me, K1, 0, CONV, 0, 512, CONV)
            _conv_matmuls(nc, mx1_ps[:, 512:LM1], mw1, me, K1, 0, CONV, 512, LM1, CONV)
            mx1 = wp.tile([CONV, LM1], F16, name="mx1")
            nc.scalar.activation(out=mx1, in_=mx1_ps, func=AF.Relu, bias=mb1[:, 0:1])

            px1_ps = ps.tile([CONV, LP1], F32, name="px1_ps", tag="ps")
            _conv_matmuls(nc, px1_ps, pw1, pe, K1, 0, CONV, 0, LP1, CONV)
            px1 = wp.tile([CONV, LP1], F16, name="px1")
            nc.scalar.activation(out=px1, in_=px1_ps, func=AF.Relu, bias=pb1[:, 0:1])

            mx2_ps = ps.tile([C2, LM2], F32, name="mx2_ps", tag="ps")
            _conv_matmuls(nc, mx2_ps[:, 0:512], mw2, mx1, K2, 0, C2, 0, 512, C2)
            _conv_matmuls(nc, mx2_ps[:, 512:LM2], mw2, mx1, K2, 0, C2, 512, LM2, C2)
            mx2 = wp.tile([C2, LM2], F16, name="mx2")
            nc.scalar.activation(out=mx2, in_=mx2_ps, func=AF.Relu, bias=mb2[:, 0:1])

            px2_ps = ps.tile([C2, LP2], F32, name="px2_ps", tag="ps")
            _conv_matmuls(nc, px2_ps, pw2, px1, K2, 0, C2, 0, LP2, C2)
            px2 = wp.tile([C2, LP2], F16, name="px2")
            nc.scalar.activation(out=px2, in_=px2_ps, func=AF.Relu, bias=pb2[:, 0:1])

            mc0_ps = ps.tile([128, LM3], F32, name="mc0_ps", tag="ps")
            _conv_matmuls(nc, mc0_ps[:, 0:512], mw3, mx2, K3, 0, 128, 0, 512, C4)
            _conv_matmuls(nc, mc0_ps[:, 512:LM3], mw3, mx2, K3, 0, 128, 512, LM3, C4)
            mc0 = wp.tile([128, LM3], F16, name="mc0")
            nc.scalar.activation(out=mc0, in_=mc0_ps, func=AF.Relu, bias=mb3[:, 0:1])
            mc1_ps = ps.tile([32, LM3], F32, name="mc1_ps", tag="ps")
            _conv_matmuls(nc, mc1_ps[:, 0:512], mw3, mx2, K3, 128, C4, 0, 512, C4)
            _conv_matmuls(nc, mc1_ps[:, 512:LM3], mw3, mx2, K3, 128, C4, 512, LM3, C4)
            mc1 = wp.tile([32, LM3], F16, name="mc1")
            nc.scalar.activation(out=mc1, in_=mc1_ps, func=AF.Relu, bias=mb3[0:32, 1:2])

            pc0_ps = ps.tile([128, LP3], F32, name="pc0_ps", tag="ps")
            _conv_matmuls(nc, pc0_ps, pw3, px2, K3, 0, 128, 0, LP3, C4)
            pc0 = wp.tile([128, LP3], F16, name="pc0")
            nc.scalar.activation(out=pc0, in_=pc0_ps, func=AF.Relu, bias=pb3[:, 0:1])
            pc1_ps = ps.tile([32, LP3], F32, name="pc1_ps", tag="ps")
            _conv_matmuls(nc, pc1_ps, pw3, px2, K3, 128, C4, 0, LP3, C4)
            pc1 = wp.tile([32, LP3], F16, name="pc1")
            nc.scalar.activation(out=pc1, in_=pc1_ps, func=AF.Relu, bias=pb3[0:32, 1:2])

            # ---- attention projections ----
            # ma[c,m] = sum_c' mc[c',m] * Wma[c',c] + bma[c]  (bf16, m padded with NEG)
            ma0_ps = ps.tile([128, LM3], F32, name="ma0_ps", tag="ps")
            nc.tensor.matmul(ma0_ps[:, 0:512], wmaa[:, 0:128], mc0[:, 0:512], start=True, stop=False)
            nc.tensor.matmul(ma0_ps[:, 0:512], wmab[:, 0:128], mc1[:, 0:512], start=False, stop=True)
            nc.tensor.matmul(ma0_ps[:, 512:LM3], wmaa[:, 0:128], mc0[:, 512:LM3], start=True, stop=False)
            nc.tensor.matmul(ma0_ps[:, 512:LM3], wmab[:, 0:128], mc1[:, 512:LM3], start=False, stop=True)
            ma0 = wp.tile([128, MPAD], F16, name="ma0")
            nc.vector.memset(ma0, NEG)
            nc.scalar.add(ma0[:, 0:LM3], ma0_ps, bma[:, 0:1])

            ma1_ps = ps.tile([32, LM3], F32, name="ma1_ps", tag="ps")
            nc.tensor.matmul(ma1_ps[:, 0:512], wmaa[:, 128:C4], mc0[:, 0:512], start=True, stop=False)
            nc.tensor.matmul(ma1_ps[:, 0:512], wmab[:, 128:C4], mc1[:, 0:512], start=False, stop=True)
            nc.tensor.matmul(ma1_ps[:, 512:LM3], wmaa[:, 128:C4], mc0[:, 512:LM3], start=True, stop=False)
            nc.tensor.matmul(ma1_ps[:, 512:LM3], wmab[:, 128:C4], mc1[:, 512:LM3], start=False, stop=True)
            ma1 = wp.tile([32, MPAD], F16, name="ma1")
            nc.vector.memset(ma1, NEG)
            nc.scalar.add(ma1[:, 0:LM3], ma1_ps, bma[0:32, 1:2])
            # replicate 4x vertically for the packed c[128:160] loop
            ma1p = wp.tile([128, MPAD], F16, name="ma1p")
            for j in range(4):
                nc.sync.dma_start(out=ma1p[32 * j:32 * (j + 1), :], in_=ma1[:, :])

            # pa[c,p] = sum_c' pc[c',p] * Wpa[c',c] + bpa[c]  (fp32: DVE bias vectors)
            pa0_ps = ps.tile([128, LP3], F32, name="pa0_ps", tag="ps")
            nc.tensor.matmul(pa0_ps, wpaa[:, 0:128], pc0, start=True, stop=False)
            nc.tensor.matmul(pa0_ps, wpab[:, 0:128], pc1, start=False, stop=True)
            pa0 = wp.tile([128, LP3], F32, name="pa0")
            nc.scalar.add(pa0, pa0_ps, bpa[:, 0:1])

            pa1_ps = ps.tile([32, LP3], F32, name="pa1_ps", tag="ps")
            nc.tensor.matmul(pa1_ps, wpaa[:, 128:C4], pc0, start=True, stop=False)
            nc.tensor.matmul(pa1_ps, wpab[:, 128:C4], pc1, start=False, stop=True)
            pa1 = wp.tile([32, 4 * NPB], F32, name="pa1")
            nc.vector.memset(pa1, NEG)
            nc.scalar.add(pa1[:, 0:LP3], pa1_ps, bpa[0:32, 1:2])
            # pack 4 p-positions per 32-row block: pa1p[32j+d, g] = pa1[d, 4g+j]
            pa1p = wp.tile([128, NPB], F32, name="pa1p")
            pa1_g = pa1.rearrange("d (g f) -> d g f", f=4)
            for j in range(4):
                nc.sync.dma_start(
                    out=pa1p[32 * j:32 * (j + 1), 0:NPB],
                    in_=pa1_g[:, :, j])

            # ---- 4D attention reductions ----
            hp0 = wp.tile([128, 88], F32, name="hp0")
            hp1p = wp.tile([128, NPB], F32, name="hp1p")
            hm0_ps = ps_hm.tile([128, MPAD], F32, name="hm0_ps")
            hm1_ps = ps_hm.tile([32, MPAD], F32, name="hm1_ps")
            zeros = wp.tile([128, MPAD], F16, name="zeros")
            nc.vector.memset(zeros, 0.0)

            def h_plane(engine, ma_t, bias_col, accum_col):
                h = hpool.tile([128, MPAD], F16, tag="h", name="h")
                if engine == 'act':
                    nc.scalar.activation(out=h, in_=ma_t, func=AF.Relu,
                                         bias=bias_col, accum_out=accum_col)
                else:
                    nc.vector.scalar_tensor_tensor(
                        out=h, in0=ma_t, scalar=bias_col, in1=zeros,
                        op0=ALU.add, op1=ALU.max, accum_out=accum_col)
                return h

            plan = _h_plan()
            n1 = sum(1 for l, _, _ in plan if l == 1)
            n2 = len(plan) - n1
            i1 = i2 = 0
            for loop_id, engine, grp in plan:
                if loop_id == 1:
                    planes = [h_plane(engine, ma0, pa0[:, p:p + 1], hp0[:, p:p + 1])
                              for p in grp]
                else:
                    planes = [h_plane(engine, ma1p, pa1p[:, g:g + 1], hp1p[:, g:g + 1])
                              for g in grp]
                if len(planes) == 2:
                    t = hppool.tile([128, MPAD], F16, tag="hs", name="hs")
                    nc.vector.tensor_tensor(out=t, in0=planes[0], in1=planes[1], op=ALU.add)
                else:
                    t = planes[0]
                if loop_id == 1:
                    nc.tensor.matmul(hm0_ps[:, 0:512], id128, t[:, 0:512],
                                     start=(i1 == 0), stop=(i1 == n1 - 1))
                    nc.tensor.matmul(hm0_ps[:, 512:MPAD], id128, t[:, 512:MPAD],
                                     start=(i1 == 0), stop=(i1 == n1 - 1))
                    i1 += 1
                else:
                    nc.tensor.matmul(hm1_ps[:, 0:512], idst, t[:, 0:512],
                                     start=(i2 == 0), stop=(i2 == n2 - 1))
                    nc.tensor.matmul(hm1_ps[:, 512:MPAD], idst, t[:, 512:MPAD],
                                     start=(i2 == 0), stop=(i2 == n2 - 1))
                    i2 += 1

            # unpack hp1p -> hp1[d, 4g+j]
            hp1 = wp.tile([32, 88], F32, name="hp1")
            hp1_g = hp1.rearrange("d (g f) -> d g f", f=4)
            for j in range(4):
                nc.sync.dma_start(
                    out=hp1_g[:, :, j],
                    in_=hp1p[32 * j:32 * j + 32, 0:NPB])

            # ---- peptide attention gate ----
            # catt[d,p] = sigmoid(sum_c hp[c,p]/LM3 * Wa[c,d] + ba[d]); 1/LM3 folded into wca
            cl0_ps = ps.tile([128, LP3], F32, name="cl0_ps", tag="ps")
            nc.tensor.matmul(cl0_ps, wcaa[:, 0:128], hp0[:, 0:LP3], start=True, stop=False)
            nc.tensor.matmul(cl0_ps, wcab[:, 0:128], hp1[:, 0:LP3], start=False, stop=True)
            catt0 = wp.tile([128, LP3], F16, name="catt0")
            nc.scalar.activation(out=catt0, in_=cl0_ps, func=AF.Sigmoid, bias=ba[:, 0:1])
            cl1_ps = ps.tile([32, LP3], F32, name="cl1_ps", tag="ps")
            nc.tensor.matmul(cl1_ps, wcaa[:, 128:C4], hp0[:, 0:LP3], start=True, stop=False)
            nc.tensor.matmul(cl1_ps, wcab[:, 128:C4], hp1[:, 0:LP3], start=False, stop=True)
            catt1 = wp.tile([32, LP3], F16, name="catt1")
            nc.scalar.activation(out=catt1, in_=cl1_ps, func=AF.Sigmoid, bias=ba[0:32, 1:2])

            pg0 = wp.tile([128, LP3], F16, name="pg0")
            nc.vector.tensor_scalar(out=catt0, in0=catt0, scalar1=0.5, scalar2=None, op0=ALU.add)
            nc.vector.tensor_tensor(out=pg0, in0=catt0, in1=pc0, op=ALU.mult)
            pv0 = wp.tile([128, 1], F16, name="pv0")
            nc.vector.tensor_reduce(out=pv0, in_=pg0, op=ALU.max, axis=AX.X)
            pg1 = wp.tile([32, LP3], F16, name="pg1")
            nc.vector.tensor_scalar(out=catt1, in0=catt1, scalar1=0.5, scalar2=None, op0=ALU.add)
            nc.vector.tensor_tensor(out=pg1, in0=catt1, in1=pc1, op=ALU.mult)
            pv1 = wp.tile([32, 1], F16, name="pv1")
            nc.vector.tensor_reduce(out=pv1, in_=pg1, op=ALU.max, axis=AX.X)

            # ---- MHC attention gate ----
            hm0 = wp.tile([128, LM3], F16, name="hm0")
            nc.scalar.copy(hm0, hm0_ps[:, 0:LM3])
            hm1 = wp.tile([32, LM3], F16, name="hm1")
            nc.scalar.copy(hm1, hm1_ps[:, 0:LM3])

            matt0 = wp.tile([128, LM3], F16, name="matt0")
            ml0_ps = ps.tile([128, LM3], F32, name="ml0_ps", tag="ps")
            for lo, hi in ((0, 512), (512, LM3)):
                nc.tensor.matmul(ml0_ps[:, lo:hi], wmaa2[:, 0:128], hm0[:, lo:hi], start=True, stop=False)
                nc.tensor.matmul(ml0_ps[:, lo:hi], wmab2[:, 0:128], hm1[:, lo:hi], start=False, stop=True)
            nc.scalar.activation(out=matt0, in_=ml0_ps, func=AF.Sigmoid, bias=ba[:, 0:1])
            matt1 = wp.tile([32, LM3], F16, name="matt1")
            ml1_ps = ps.tile([32, LM3], F32, name="ml1_ps", tag="ps")
            for lo, hi in ((0, 512), (512, LM3)):
                nc.tensor.matmul(ml1_ps[:, lo:hi], wmaa2[:, 128:C4], hm0[:, lo:hi], start=True, stop=False)
                nc.tensor.matmul(ml1_ps[:, lo:hi], wmab2[:, 128:C4], hm1[:, lo:hi], start=False, stop=True)
            nc.scalar.activation(out=matt1, in_=ml1_ps, func=AF.Sigmoid, bias=ba[0:32, 1:2])

            mg0 = wp.tile([128, LM3], F16, name="mg0")
            nc.vector.tensor_scalar(out=matt0, in0=matt0, scalar1=0.5, scalar2=None, op0=ALU.add)
            nc.vector.tensor_tensor(out=mg0, in0=matt0, in1=mc0, op=ALU.mult)
            mv0 = wp.tile([128, 1], F16, name="mv0")
            nc.vector.tensor_reduce(out=mv0, in_=mg0, op=ALU.max, axis=AX.X)
            mg1 = wp.tile([32, LM3], F16, name="mg1")
            nc.vector.tensor_scalar(out=matt1, in0=matt1, scalar1=0.5, scalar2=None, op0=ALU.add)
            nc.vector.tensor_tensor(out=mg1, in0=matt1, in1=mc1, op=ALU.mult)
            mv1 = wp.tile([32, 1], F16, name="mv1")
            nc.vector.tensor_reduce(out=mv1, in_=mg1, op=ALU.max, axis=AX.X)

            # ---- FC head ----
            # out_row[1, N] = x.T @ W accumulated over k-chunks, bias via
            # ones-stationary matmul, Lrelu fused into PSUM->SBUF, then
            # transposed to [128, N/128] by 1-row-stationary matmuls.
            def fc_row(name, chunks, bias_row, n_out):
                """chunks: list of (x_chunk [k,1] bf16, w_ap [k, n_out] bf16)."""
                row_ps = ps.tile([1, n_out], F32, name=name + "_ps", tag="ps")
                for lo in range(0, n_out, 512):
                    hi = min(lo + 512, n_out)
                    n = len(chunks)
                    for i, (xc, wc) in enumerate(chunks):
                        nc.tensor.matmul(row_ps[:, lo:hi], xc, wc[:, lo:hi],
                                         start=(i == 0), stop=False)
                    nc.tensor.matmul(row_ps[:, lo:hi], one_bf, bias_row[:, lo:hi],
                                     start=False, stop=True)
                return row_ps

            def fc_transpose(name, row_sb, n_out):
                nblk = n_out // 128
                col_ps = ps.tile([128, nblk], F32, name=name + "_cps", tag="ps")
                for b in range(nblk):
                    nc.tensor.matmul(col_ps[:, b:b + 1],
                                     row_sb[0:1, 128 * b:128 * (b + 1)],
                                     one_bf, start=True, stop=True)
                col = wp.tile([128, nblk], F16, name=name + "_col")
                nc.scalar.copy(col, col_ps)
                return col

            f1_ps = fc_row("f1", [(pv0, w1a[:, 0:1024]), (pv1, w1b[:, 0:1024]),
                                  (mv0, w1a[:, 1024:2048]), (mv1, w1b[:, 1024:2048])],
                           b1r, 1024)
            f1row = wp.tile([1, 1024], F16, name="f1row")
            nc.scalar.activation(out=f1row, in_=f1_ps, func=AF.Lrelu, alpha=0.01)
            x1 = fc_transpose("f1", f1row, 1024)

            f2_ps = fc_row("f2", [(x1[:, c:c + 1], w2[:, 1024 * c:1024 * (c + 1)])
                                  for c in range(8)], b2r, 1024)
            f2row = wp.tile([1, 1024], F16, name="f2row")
            nc.scalar.activation(out=f2row, in_=f2_ps, func=AF.Lrelu, alpha=0.01)
            x2 = fc_transpose("f2", f2row, 1024)

            f3_ps = fc_row("f3", [(x2[:, c:c + 1], w3[:, 512 * c:512 * (c + 1)])
                                  for c in range(8)], b3r, 512)
            f3row = wp.tile([1, 512], F16, name="f3row")
            nc.scalar.activation(out=f3row, in_=f3_ps, func=AF.Lrelu, alpha=0.01)
            x3 = fc_transpose("f3", f3row, 512)

            o_ps = ps.tile([1, 2], F32, name="o_ps", tag="ps")
            for c in range(4):
                nc.tensor.matmul(o_ps, x3[:, c:c + 1], wo[:, 2 * c:2 * c + 2],
                                 start=(c == 0), stop=(c == 3))
            o_lin = wp.tile([1, 2], F32, name="o_lin")
            nc.scalar.copy(o_lin, o_ps)
            o_sb = wp.tile([1, 2], F32, name="o_sb")
            nc.vector.tensor_tensor(out=o_sb, in0=o_lin, in1=bo, op=ALU.add)
            nc.sync.dma_start(out=out_e[:], in_=o_sb)

    _split_excess_waits(nc, max_waits=1)
    return nc


_PROGRAM = None


def _get_program():
    global _PROGRAM
    if _PROGRAM is None:
        _PROGRAM = _build_program()
    return _PROGRAM


def _prep_weights(inp):
    """Host-side layout prep shared by all cores."""
    f = lambda x: np.ascontiguousarray(np.asarray(x, dtype=np.float32))
    bf = lambda x: np.ascontiguousarray(np.asarray(x, dtype=np.float32).astype(np.float16))

    def convw(w):  # [co, ci, k] -> [ci, k*co] bf16
        w = np.asarray(w, dtype=np.float32)
        ci = w.shape[1]
        return np.ascontiguousarray(
            w.transpose(1, 2, 0).reshape(ci, -1).astype(np.float16))

    def bias2(b):  # [160] -> [128, 2] (col 0 = [0:128], col 1 rows 0:32 = [128:160])
        b = np.asarray(b, dtype=np.float32)
        out = np.zeros((128, 2), np.float32)
        out[:, 0] = b[0:128]
        out[0:32, 1] = b[128:160]
        return out

    def fcw(w, nblk):  # [I, J] with I = nblk*128 -> [128, nblk*J] bf16
        w = np.asarray(w, dtype=np.float32)
        i, j = w.shape
        return np.ascontiguousarray(
            w.reshape(nblk, 128, j).transpose(1, 0, 2).reshape(128, nblk * j)
            .astype(np.float16))

    wa985 = np.asarray(inp['Wa'], np.float32) / float(LM3)
    wa85 = np.asarray(inp['Wa'], np.float32) / float(LP3)
    w1 = np.asarray(inp['W1'], np.float32)
    d = {
        'pep_emb': bf(inp['pep_emb']), 'mhc_emb': bf(inp['mhc_emb']),
        'pw1t': convw(inp['pw1']), 'pw2t': convw(inp['pw2']), 'pw3t': convw(inp['pw3']),
        'mw1t': convw(inp['mw1']), 'mw2t': convw(inp['mw2']), 'mw3t': convw(inp['mw3']),
        'pb1': f(inp['pb1']).reshape(CONV, 1), 'pb2': f(inp['pb2']).reshape(C2, 1), 'pb3': bias2(inp['pb3']),
        'mb1': f(inp['mb1']).reshape(CONV, 1), 'mb2': f(inp['mb2']).reshape(C2, 1), 'mb3': bias2(inp['mb3']),
        'wpa_a': bf(inp['Wpa'][0:128]), 'wpa_b': bf(inp['Wpa'][128:160]),
        'wma_a': bf(inp['Wma'][0:128]), 'wma_b': bf(inp['Wma'][128:160]),
        'wca_a': f(wa985[0:128]), 'wca_b': f(wa985[128:160]),
        'wma2_a': bf(wa85[0:128]), 'wma2_b': bf(wa85[128:160]),
        'bpa': bias2(inp['bpa']), 'bma': bias2(inp['bma']), 'ba': bias2(inp['ba']),
        'w1a': np.ascontiguousarray(
            np.concatenate([w1[0:128], w1[160:288]], axis=1).astype(np.float16)),
        'w1b': np.ascontiguousarray(
            np.concatenate([w1[128:160], w1[288:320]], axis=1).astype(np.float16)),
        'w2': fcw(inp['W2'], 8), 'w3': fcw(inp['W3'], 8),
        'wo': fcw(inp['Wo'], 4),
        'b1row': bf(inp['b1']).reshape(1, 1024), 'b2row': bf(inp['b2']).reshape(1, 1024),
        'b3row': bf(inp['b3']).reshape(1, 512),
        'bo': f(inp['bo']).reshape(1, 2),
        'ident128': np.eye(128, dtype=np.float16),
        'ident_st': np.ascontiguousarray(np.tile(np.eye(32, dtype=np.float16), (4, 1))),
    }
    return d


def _onehot(idx, length):
    idx = np.asarray(idx).astype(np.int64)
    oh = np.zeros((26, length), np.float32)
    oh[idx, np.arange(length)] = 1.0
    return np.ascontiguousarray(oh.astype(np.float16))


def kernel(**inputs):
    nc = _get_program()
    shared = _prep_weights(inputs)
    peptide = np.asarray(inputs['peptide'])
    mhc = np.asarray(inputs['MHC'])
    in_maps = []
    for b in range(B):
        m = dict(shared)
        m['pep_oh'] = _onehot(peptide[b], LP)
        m['mhc_oh'] = _onehot(mhc[b], LM)
        in_maps.append(m)
    res = run_bass_kernel_spmd(nc, in_maps, core_ids=list(range(B)))
    return np.stack([np.asarray(res.results[i]['out']).reshape(2) for i in range(B)]).astype(np.float32)
